# revision 29
# baseline (speedup 1.0000x reference)
"""MoE (base FFN + top-2-of-8 expert FFNs) on 8 TRN2 NeuronCores.

Strategy (paired expert-parallel):
  - Routing (softmax over 8 experts, top-2, renormalize) is computed on
    host with jax-CPU, mirroring the reference computation exactly.
  - Experts are sorted by token count and paired big-with-small; pair p
    lives on cores {2p, 2p+1}.  Each expert's tokens are split in half
    across its two host cores, so per-core expert capacity is
    SA + SB = ceil(max_big/2) + ceil(max_small/2)  (~1064 tokens)
    instead of max_e n_e (~1152) — every core does the same, smaller,
    amount of matmul work.
  - Each core also runs the base FFN for tokens [512e, 512(e+1)).
  - Host scatters expert outputs back (scatter-add) on top of base.

Device compute in bf16 with fp32 PSUM accumulation; activations stay in
[feature, token] layout so both FFN matmuls chain without transposes.
Weight-tile loads for the two expert segments are issued from separate
engines (scalar / gpsimd) so no DMA stream head-of-line-blocks another.
Warm-up junk matmuls (memset tiles prepared on the vector engine, which
exits the preamble first) keep the PE HAM clock warm while the first
activation loads are in flight.
"""

import numpy as np
import ml_dtypes

import concourse.bass as bass
import concourse.mybir as mybir
import concourse.tile as tile
from concourse import bacc
from concourse.bass_utils import run_bass_kernel_spmd
from concourse.tile_rust import add_dep_helper

P = 128
B, S, H, F, E = 2, 2048, 1024, 4096, 8
T = B * S
NB = T // 8  # base-FFN tokens per core
TOP_K = 2
BETA = 1.0

F32 = mybir.dt.float32
CDT = mybir.dt.bfloat16  # compute dtype on the tensor engine
NP_CDT = ml_dtypes.bfloat16

KA = H // P   # 8  k-subtiles contracting H
FB = F // P   # 32 output blocks of F
KB = F // P   # 32 k-subtiles contracting F
HB = H // P   # 8  output blocks of H
CHUNK = 512   # matmul moving free dim / PSUM bank width


def _seg_chunks(off, n):
    # near-equal even-sized chunks: every matmul stays long enough
    # (>=236 cols) to hide the next LDWEIGHTS behind it
    k = (n + CHUNK - 1) // CHUNK
    out = []
    c0 = 0
    for i in range(k):
        rem, left = n - c0, k - i
        cn = min((((rem + left - 1) // left + 1) // 2) * 2, rem)
        out.append((off + c0, cn))
        c0 += cn
    return out


def _stage(nc, wpool, pspool, wtag, segs, x_s, evict, wt0s=None, gate=None):
    """One matmul stage over weight segments.

    segs: list of (w_dram [P, OB, K, 128], chunk list [(c0, cn), ...]).
    All segments share OB/K; segment si's chunks contract with weight si.
    Weight tiles for segment 0 load via scalar, segment 1 via gpsimd.
    gate: optional (marker_inst, n_first) — weight loads for the first
    n_first obs wait on marker, keeping them off the startup window.
    """
    OB, K = segs[0][0].shape[1], segs[0][0].shape[2]
    engs = [nc.scalar, nc.gpsimd]
    for ob in range(OB):
        wts = []
        for si, (w_d, _) in enumerate(segs):
            if ob == 0 and wt0s is not None and wt0s[si] is not None:
                wts.append(wt0s[si])
                continue
            # single rotation name: pool recycling then naturally defers
            # prefetch of later stages' weights behind the current stage
            wt = wpool.tile([P, K, P], CDT, name=wtag)
            dma = engs[si % 2].dma_start(out=wt[:], in_=w_d[:, ob])
            if gate is not None and ob < gate[1]:
                add_dep_helper(dma.ins, gate[0].ins,
                               reason="defer prefetch past startup window")
            wts.append(wt)
        pss = []
        for si, (_, cts) in enumerate(segs):
            for c0, cn in cts:
                pss.append((si, pspool.tile([P, CHUNK], F32, name="ps"), c0, cn))
        for k in range(K):
            for si, ps, c0, cn in pss:
                nc.tensor.matmul(
                    ps[:, :cn],
                    wts[si][:, k],
                    x_s[:, k, c0 : c0 + cn],
                    start=(k == 0),
                    stop=(k == K - 1),
                )
        for _, ps, c0, cn in pss:
            evict(ob, ps, c0, cn)


def _build(SA, SB):
    """Per-core SPMD program: base FFN (NB tokens) + two expert segments
    of capacity SA and SB."""
    C = SA + SB
    nc = bacc.Bacc(None, target_bir_lowering=False, debug=False)
    act_silu = mybir.ActivationFunctionType.Silu
    with tile.TileContext(nc) as tc:
        with tc.tile_pool(name="dram", bufs=1, space="DRAM") as dram:
            kw = dict(kind="ExternalInput", uniquify=False)
            xg = dram.tile((P, KA, C), CDT, name="xg", **kw)
            wg = dram.tile((P, C), F32, name="wg", **kw)
            w1a = dram.tile((P, FB, KA, P), CDT, name="w1a", **kw)
            w1b = dram.tile((P, FB, KA, P), CDT, name="w1b", **kw)
            w2a = dram.tile((P, HB, KB, P), CDT, name="w2a", **kw)
            w2b = dram.tile((P, HB, KB, P), CDT, name="w2b", **kw)
            xb = dram.tile((P, KA, NB), CDT, name="xb", **kw)
            b1 = dram.tile((P, FB, KA, P), CDT, name="b1", **kw)
            b2 = dram.tile((P, HB, KB, P), CDT, name="b2", **kw)
            ymoe = dram.tile(
                (P, HB, C), CDT, name="ymoe", kind="ExternalOutput", uniquify=False
            )
            ybase = dram.tile(
                (P, HB, NB), CDT, name="ybase", kind="ExternalOutput", uniquify=False
            )
            with (
                tc.tile_pool(name="res", bufs=1) as res,
                tc.tile_pool(name="wa", bufs=12) as wa,
                tc.tile_pool(name="wb", bufs=5) as wb,
                tc.tile_pool(name="ps", bufs=8, space="PSUM") as ps,
                tc.tile_pool(name="yo", bufs=4) as yo,
            ):
                cts_a = _seg_chunks(0, SA)
                cts_b = _seg_chunks(SA, SB)
                cts_base = _seg_chunks(0, NB)

                # PE warm-up: junk matmuls on zeroed scratch tiles while the
                # first real loads are in flight, so the HAM clock gate is
                # already at full rate when real matmuls start.  Memsets go
                # on the vector engine — it clears the framework preamble
                # first, so the junk matmuls start as early as possible.
                wlhs = res.tile([P, P], CDT, name="wlhs")
                nc.vector.memset(wlhs[:], 0.0)
                wrhs = res.tile([P, CHUNK], CDT, name="wrhs")
                nc.vector.memset(wrhs[:], 0.0)
                wps = ps.tile([P, CHUNK], F32, name="ps")
                for _ in range(11):
                    nc.tensor.matmul(wps[:], wlhs[:], wrhs[:], start=True, stop=True)

                # Base FFN first: tiny input loads -> PE starts fast, and
                # the big expert-token load (xg) hides under base compute.
                # xb split across two DMA queues so the first k-subtiles
                # land quickly and base1 can start.
                wt_b1 = wa.tile([P, KA, P], CDT, name="wa")
                nc.scalar.dma_start(out=wt_b1[:], in_=b1[:, 0])
                xb_s = res.tile([P, KA, NB], CDT, name="xb_s")
                xb_engs = [nc.sync, nc.scalar, nc.gpsimd]
                for i, (k0, k1) in enumerate(((0, 3), (3, 6), (6, 8))):
                    xb_engs[i].dma_start(out=xb_s[:, k0:k1], in_=xb[:, k0:k1])

                h2 = res.tile([P, KB, NB], CDT, name="h2")

                base1_marker = {}

                def ev_base1(ob, psum, c0, cn):
                    act = nc.scalar.activation(
                        h2[:, ob, c0 : c0 + cn], psum[:, :cn], act_silu
                    )
                    base1_marker[ob] = act

                _stage(nc, wa, ps, "wa", [(b1, cts_base)], xb_s, ev_base1,
                       wt0s=[wt_b1])

                # expert tokens: loaded during base compute; each piece is
                # gated on base1 progress so the 4MB transfer is spread out
                # and never starves the startup-critical xb/b1 loads
                xg_s = res.tile([P, KA, C], CDT, name="xg_s")
                for j, k in enumerate(range(0, KA, 2)):
                    dma = nc.sync.dma_start(out=xg_s[:, k : k + 2], in_=xg[:, k : k + 2])
                    add_dep_helper(
                        dma.ins,
                        base1_marker[2 + 4 * j].ins,
                        reason="spread xg load across base1",
                    )

                out_engines = [nc.gpsimd, nc.sync, nc.scalar]
                ev_n = [0]

                def _out_dma(dst, o, cn):
                    eng = out_engines[ev_n[0] % 3]
                    ev_n[0] += 1
                    eng.dma_start(out=dst, in_=o[:, :cn])

                def ev_base2(ob, psum, c0, cn):
                    o = yo.tile([P, CHUNK], CDT, name="yo")
                    nc.vector.tensor_copy(out=o[:, :cn], in_=psum[:, :cn])
                    _out_dma(ybase[:, ob, c0 : c0 + cn], o, cn)

                _stage(nc, wb, ps, "wb", [(b2, cts_base)], h2, ev_base2,
                       gate=(base1_marker[16], 4))

                h1 = res.tile([P, KB, C], CDT, name="h1")

                def ev_moe1(ob, psum, c0, cn):
                    nc.scalar.activation(
                        h1[:, ob, c0 : c0 + cn], psum[:, :cn], act_silu
                    )

                _stage(nc, wa, ps, "wa", [(w1a, cts_a), (w1b, cts_b)], xg_s, ev_moe1)

                wg_s = res.tile([P, C], F32, name="wg_s")
                dma = nc.sync.dma_start(out=wg_s[:], in_=wg[:])
                add_dep_helper(dma.ins, base1_marker[20].ins,
                               reason="defer wg load past startup window")

                def ev_moe2(ob, psum, c0, cn):
                    o = yo.tile([P, CHUNK], CDT, name="yo")
                    nc.vector.tensor_tensor(
                        out=o[:, :cn],
                        in0=psum[:, :cn],
                        in1=wg_s[:, c0 : c0 + cn],
                        op=mybir.AluOpType.mult,
                    )
                    _out_dma(ymoe[:, ob, c0 : c0 + cn], o, cn)

                _stage(nc, wb, ps, "wb", [(w2a, cts_a), (w2b, cts_b)], h1, ev_moe2)
    nc.compile()
    return nc


_BUILD_CACHE = {}


def _get_program(SA, SB):
    if (SA, SB) not in _BUILD_CACHE:
        _BUILD_CACHE[(SA, SB)] = _build(SA, SB)
    return _BUILD_CACHE[(SA, SB)]


def _routing(x, router_w):
    """Replicate the reference router bit-for-bit on jax CPU."""
    import jax
    import jax.numpy as jnp

    cpu = jax.devices("cpu")[0]

    def _route(xj, rj):
        logits = xj @ rj
        probs = jax.nn.softmax(logits, axis=-1)
        top_w, top_i = jax.lax.top_k(probs, TOP_K)
        top_w = top_w / jnp.sum(top_w, axis=-1, keepdims=True)
        return top_w, top_i

    with jax.default_device(cpu):
        top_w, top_i = jax.jit(_route)(jnp.asarray(x), jnp.asarray(router_w))
        top_w = np.asarray(top_w)
        top_i = np.asarray(top_i)
    return top_w, top_i


def _as_pkc(w, kb, nb):
    # [K, N] -> [P, nblocks, kblocks, 128]: w[k*128+p, n*128+c] -> [p, n, k, c]
    return np.ascontiguousarray(w.reshape(kb, P, nb, P).transpose(1, 2, 0, 3))


def _as_pit(xt):
    # [R, N] -> [P, R//128, N]: xt[i*128+p, t] -> [p, i, t]
    r, n = xt.shape
    return np.ascontiguousarray(xt.reshape(r // P, P, n).transpose(1, 0, 2))


def _from_pit(y):
    # [P, R//128, N] -> [N, R]
    p, i, n = y.shape
    return y.transpose(2, 1, 0).reshape(n, i * p)


def _r2(n):
    return max(((n + 1) // 2) * 2, 8)


def kernel(hidden_states, router_w, base_w1, base_w2, exp_w1, exp_w2):
    x = np.ascontiguousarray(hidden_states.reshape(T, H), dtype=np.float32)
    top_w, top_i = _routing(x, np.asarray(router_w, dtype=np.float32))

    # per-expert token lists
    idx = []
    wts = []
    for e in range(E):
        rows, slots = np.nonzero(top_i == e)
        idx.append(rows)
        wts.append(top_w[rows, slots].astype(np.float32))
    n_e = np.array([len(r) for r in idx])

    # pair the biggest experts with the smallest: pair p -> cores {2p, 2p+1}
    order = np.argsort(-n_e, kind="stable")
    bigs = order[: E // 2]
    smalls = order[E // 2 :][::-1]
    SA = _r2((int(n_e[bigs].max()) + 1) // 2)
    SB = _r2((int(n_e[smalls].max()) + 1) // 2)
    C = SA + SB

    nc = _get_program(SA, SB)

    xT_c = np.ascontiguousarray(x.T).astype(NP_CDT)  # [H, T]

    b1_dev = _as_pkc(np.asarray(base_w1, np.float32).astype(NP_CDT), KA, FB)
    b2_dev = _as_pkc(np.asarray(base_w2, np.float32).astype(NP_CDT), KB, HB)
    w1_dev = [
        _as_pkc(np.asarray(exp_w1[e], np.float32).astype(NP_CDT), KA, FB)
        for e in range(E)
    ]
    w2_dev = [
        _as_pkc(np.asarray(exp_w2[e], np.float32).astype(NP_CDT), KB, HB)
        for e in range(E)
    ]

    # expert halves: expert e's tokens split between its two host cores
    parts = {}  # (pair, half) -> (big_rows, small_rows)
    for p in range(E // 2):
        ia, ib = idx[bigs[p]], idx[smalls[p]]
        ha, hb = (len(ia) + 1) // 2, (len(ib) + 1) // 2
        parts[(p, 0)] = (ia[:ha], ib[:hb])
        parts[(p, 1)] = (ia[ha:], ib[hb:])
    wmap = {}
    for p in range(E // 2):
        wa_, wb_ = wts[bigs[p]], wts[smalls[p]]
        ha, hb = (len(wa_) + 1) // 2, (len(wb_) + 1) // 2
        wmap[(p, 0)] = (wa_[:ha], wb_[:hb])
        wmap[(p, 1)] = (wa_[ha:], wb_[hb:])

    in_maps = []
    for c in range(E):
        p, half = c // 2, c % 2
        ra, rb = parts[(p, half)]
        va, vb = wmap[(p, half)]
        xg_full = np.zeros((H, C), dtype=NP_CDT)
        xg_full[:, : len(ra)] = xT_c[:, ra]
        xg_full[:, SA : SA + len(rb)] = xT_c[:, rb]
        wg_full = np.zeros((C,), dtype=np.float32)
        wg_full[: len(va)] = va
        wg_full[SA : SA + len(vb)] = vb
        in_maps.append(
            {
                "xg": _as_pit(xg_full),
                "wg": np.ascontiguousarray(np.broadcast_to(wg_full, (P, C))),
                "w1a": w1_dev[bigs[p]],
                "w1b": w1_dev[smalls[p]],
                "w2a": w2_dev[bigs[p]],
                "w2b": w2_dev[smalls[p]],
                "xb": _as_pit(xT_c[:, c * NB : (c + 1) * NB]),
                "b1": b1_dev,
                "b2": b2_dev,
            }
        )

    res = run_bass_kernel_spmd(nc, in_maps, core_ids=list(range(8)))

    out = np.empty((T, H), dtype=np.float32)
    for c in range(E):
        out[c * NB : (c + 1) * NB] = _from_pit(res.results[c]["ybase"]).astype(
            np.float32
        )
    for c in range(E):
        p, half = c // 2, c % 2
        ra, rb = parts[(p, half)]
        ym = _from_pit(res.results[c]["ymoe"]).astype(np.float32)
        if len(ra):
            out[ra] += BETA * ym[: len(ra)]
        if len(rb):
            out[rb] += BETA * ym[SA : SA + len(rb)]
    return out.reshape(B, S, H)


# revision 30
# speedup vs baseline: 1.0068x; 1.0068x over previous
"""MoE (base FFN + top-2-of-8 expert FFNs) on 8 TRN2 NeuronCores.

Strategy (paired expert-parallel):
  - Routing (softmax over 8 experts, top-2, renormalize) is computed on
    host with jax-CPU, mirroring the reference computation exactly.
  - Experts are sorted by token count and paired big-with-small; pair p
    lives on cores {2p, 2p+1}.  Each expert's tokens are split in half
    across its two host cores, so per-core expert capacity is
    SA + SB = ceil(max_big/2) + ceil(max_small/2)  (~1064 tokens)
    instead of max_e n_e (~1152) — every core does the same, smaller,
    amount of matmul work.
  - Each core also runs the base FFN for tokens [512e, 512(e+1)).
  - Host scatters expert outputs back (scatter-add) on top of base.

Device compute in bf16 with fp32 PSUM accumulation; activations stay in
[feature, token] layout so both FFN matmuls chain without transposes.
Weight-tile loads for the two expert segments are issued from separate
engines (scalar / gpsimd) so no DMA stream head-of-line-blocks another.
Warm-up junk matmuls (memset tiles prepared on the vector engine, which
exits the preamble first) keep the PE HAM clock warm while the first
activation loads are in flight.
"""

import numpy as np
import ml_dtypes

import concourse.bass as bass
import concourse.mybir as mybir
import concourse.tile as tile
from concourse import bacc
from concourse.bass_utils import run_bass_kernel_spmd
from concourse.tile_rust import add_dep_helper

P = 128
B, S, H, F, E = 2, 2048, 1024, 4096, 8
T = B * S
NB = T // 8  # base-FFN tokens per core
TOP_K = 2
BETA = 1.0

F32 = mybir.dt.float32
CDT = mybir.dt.bfloat16  # compute dtype on the tensor engine
NP_CDT = ml_dtypes.bfloat16

KA = H // P   # 8  k-subtiles contracting H
FB = F // P   # 32 output blocks of F
KB = F // P   # 32 k-subtiles contracting F
HB = H // P   # 8  output blocks of H
CHUNK = 512   # matmul moving free dim / PSUM bank width


def _seg_chunks(off, n):
    # near-equal even-sized chunks: every matmul stays long enough
    # (>=236 cols) to hide the next LDWEIGHTS behind it
    k = (n + CHUNK - 1) // CHUNK
    out = []
    c0 = 0
    for i in range(k):
        rem, left = n - c0, k - i
        cn = min((((rem + left - 1) // left + 1) // 2) * 2, rem)
        out.append((off + c0, cn))
        c0 += cn
    return out


def _stage(nc, wpool, pspool, wtag, segs, x_s, evict, wt0s=None, gate=None):
    """One matmul stage over weight segments.

    segs: list of (w_dram [P, OB, K, 128], chunk list [(c0, cn), ...]).
    All segments share OB/K; segment si's chunks contract with weight si.
    Weight tiles for segment 0 load via scalar, segment 1 via gpsimd.
    gate: optional (marker_inst, n_first) — weight loads for the first
    n_first obs wait on marker, keeping them off the startup window.
    """
    OB, K = segs[0][0].shape[1], segs[0][0].shape[2]
    engs = [nc.scalar, nc.gpsimd]
    for ob in range(OB):
        wts = []
        for si, (w_d, _) in enumerate(segs):
            if ob == 0 and wt0s is not None and wt0s[si] is not None:
                wts.append(wt0s[si])
                continue
            # single rotation name: pool recycling then naturally defers
            # prefetch of later stages' weights behind the current stage
            wt = wpool.tile([P, K, P], CDT, name=wtag)
            dma = engs[si % 2].dma_start(out=wt[:], in_=w_d[:, ob])
            if gate is not None and ob < gate[1]:
                add_dep_helper(dma.ins, gate[0].ins,
                               reason="defer prefetch past startup window")
            wts.append(wt)
        pss = []
        for si, (_, cts) in enumerate(segs):
            for c0, cn in cts:
                pss.append((si, pspool.tile([P, CHUNK], F32, name="ps"), c0, cn))
        for k in range(K):
            for si, ps, c0, cn in pss:
                nc.tensor.matmul(
                    ps[:, :cn],
                    wts[si][:, k],
                    x_s[:, k, c0 : c0 + cn],
                    start=(k == 0),
                    stop=(k == K - 1),
                )
        for _, ps, c0, cn in pss:
            evict(ob, ps, c0, cn)


def _build(SA, SB):
    """Per-core SPMD program: base FFN (NB tokens) + two expert segments
    of capacity SA and SB."""
    C = SA + SB
    nc = bacc.Bacc(None, target_bir_lowering=False, debug=False)
    act_silu = mybir.ActivationFunctionType.Silu
    with tile.TileContext(nc) as tc:
        with tc.tile_pool(name="dram", bufs=1, space="DRAM") as dram:
            kw = dict(kind="ExternalInput", uniquify=False)
            xg = dram.tile((P, KA, C), CDT, name="xg", **kw)
            wg = dram.tile((P, C), F32, name="wg", **kw)
            w1a = dram.tile((P, FB, KA, P), CDT, name="w1a", **kw)
            w1b = dram.tile((P, FB, KA, P), CDT, name="w1b", **kw)
            w2a = dram.tile((P, HB, KB, P), CDT, name="w2a", **kw)
            w2b = dram.tile((P, HB, KB, P), CDT, name="w2b", **kw)
            xb = dram.tile((P, KA, NB), CDT, name="xb", **kw)
            b1 = dram.tile((P, FB, KA, P), CDT, name="b1", **kw)
            b2 = dram.tile((P, HB, KB, P), CDT, name="b2", **kw)
            ymoe = dram.tile(
                (P, HB, C), F32, name="ymoe", kind="ExternalOutput", uniquify=False
            )
            ybase = dram.tile(
                (P, HB, NB), F32, name="ybase", kind="ExternalOutput", uniquify=False
            )
            with (
                tc.tile_pool(name="res", bufs=1) as res,
                tc.tile_pool(name="wa", bufs=12) as wa,
                tc.tile_pool(name="wb", bufs=5) as wb,
                tc.tile_pool(name="ps", bufs=8, space="PSUM") as ps,
                tc.tile_pool(name="yo", bufs=4) as yo,
            ):
                cts_a = _seg_chunks(0, SA)
                cts_b = _seg_chunks(SA, SB)
                cts_base = _seg_chunks(0, NB)

                # PE warm-up: junk matmuls on zeroed scratch tiles while the
                # first real loads are in flight, so the HAM clock gate is
                # already at full rate when real matmuls start.  Memsets go
                # on the vector engine — it clears the framework preamble
                # first, so the junk matmuls start as early as possible.
                wlhs = res.tile([P, P], CDT, name="wlhs")
                nc.vector.memset(wlhs[:], 0.0)
                wrhs = res.tile([P, CHUNK], CDT, name="wrhs")
                nc.vector.memset(wrhs[:], 0.0)
                wps = ps.tile([P, CHUNK], F32, name="ps")
                for _ in range(12):
                    nc.tensor.matmul(wps[:], wlhs[:], wrhs[:], start=True, stop=True)

                # Base FFN first: tiny input loads -> PE starts fast, and
                # the big expert-token load (xg) hides under base compute.
                # xb split across two DMA queues so the first k-subtiles
                # land quickly and base1 can start.
                wt_b1 = wa.tile([P, KA, P], CDT, name="wa")
                nc.scalar.dma_start(out=wt_b1[:], in_=b1[:, 0])
                xb_s = res.tile([P, KA, NB], CDT, name="xb_s")
                xb_engs = [nc.sync, nc.scalar, nc.gpsimd]
                for k in range(KA):
                    xb_engs[k % 3].dma_start(out=xb_s[:, k : k + 1], in_=xb[:, k : k + 1])

                h2 = res.tile([P, KB, NB], CDT, name="h2")

                base1_marker = {}

                def ev_base1(ob, psum, c0, cn):
                    act = nc.scalar.activation(
                        h2[:, ob, c0 : c0 + cn], psum[:, :cn], act_silu
                    )
                    base1_marker[ob] = act

                _stage(nc, wa, ps, "wa", [(b1, cts_base)], xb_s, ev_base1,
                       wt0s=[wt_b1])

                # expert tokens: loaded during base compute; each piece is
                # gated on base1 progress so the 4MB transfer is spread out
                # and never starves the startup-critical xb/b1 loads
                xg_s = res.tile([P, KA, C], CDT, name="xg_s")
                for j, k in enumerate(range(0, KA, 2)):
                    dma = nc.sync.dma_start(out=xg_s[:, k : k + 2], in_=xg[:, k : k + 2])
                    add_dep_helper(
                        dma.ins,
                        base1_marker[2 + 4 * j].ins,
                        reason="spread xg load across base1",
                    )

                out_engines = [nc.gpsimd, nc.sync, nc.scalar]
                ev_n = [0]

                def _out_dma(dst, o, cn):
                    eng = out_engines[ev_n[0] % 3]
                    ev_n[0] += 1
                    eng.dma_start(out=dst, in_=o[:, :cn])

                def ev_base2(ob, psum, c0, cn):
                    o = yo.tile([P, CHUNK], F32, name="yo")
                    nc.vector.tensor_copy(out=o[:, :cn], in_=psum[:, :cn])
                    _out_dma(ybase[:, ob, c0 : c0 + cn], o, cn)

                _stage(nc, wb, ps, "wb", [(b2, cts_base)], h2, ev_base2,
                       gate=(base1_marker[16], 4))

                h1 = res.tile([P, KB, C], CDT, name="h1")

                def ev_moe1(ob, psum, c0, cn):
                    nc.scalar.activation(
                        h1[:, ob, c0 : c0 + cn], psum[:, :cn], act_silu
                    )

                _stage(nc, wa, ps, "wa", [(w1a, cts_a), (w1b, cts_b)], xg_s, ev_moe1)

                wg_s = res.tile([P, C], F32, name="wg_s")
                dma = nc.sync.dma_start(out=wg_s[:], in_=wg[:])
                add_dep_helper(dma.ins, base1_marker[20].ins,
                               reason="defer wg load past startup window")

                def ev_moe2(ob, psum, c0, cn):
                    o = yo.tile([P, CHUNK], F32, name="yo")
                    nc.vector.tensor_tensor(
                        out=o[:, :cn],
                        in0=psum[:, :cn],
                        in1=wg_s[:, c0 : c0 + cn],
                        op=mybir.AluOpType.mult,
                    )
                    _out_dma(ymoe[:, ob, c0 : c0 + cn], o, cn)

                _stage(nc, wb, ps, "wb", [(w2a, cts_a), (w2b, cts_b)], h1, ev_moe2)
    nc.compile()
    return nc


_BUILD_CACHE = {}


def _get_program(SA, SB):
    if (SA, SB) not in _BUILD_CACHE:
        _BUILD_CACHE[(SA, SB)] = _build(SA, SB)
    return _BUILD_CACHE[(SA, SB)]


def _routing(x, router_w):
    """Replicate the reference router bit-for-bit on jax CPU."""
    import jax
    import jax.numpy as jnp

    cpu = jax.devices("cpu")[0]

    def _route(xj, rj):
        logits = xj @ rj
        probs = jax.nn.softmax(logits, axis=-1)
        top_w, top_i = jax.lax.top_k(probs, TOP_K)
        top_w = top_w / jnp.sum(top_w, axis=-1, keepdims=True)
        return top_w, top_i

    with jax.default_device(cpu):
        top_w, top_i = jax.jit(_route)(jnp.asarray(x), jnp.asarray(router_w))
        top_w = np.asarray(top_w)
        top_i = np.asarray(top_i)
    return top_w, top_i


def _as_pkc(w, kb, nb):
    # [K, N] -> [P, nblocks, kblocks, 128]: w[k*128+p, n*128+c] -> [p, n, k, c]
    return np.ascontiguousarray(w.reshape(kb, P, nb, P).transpose(1, 2, 0, 3))


def _as_pit(xt):
    # [R, N] -> [P, R//128, N]: xt[i*128+p, t] -> [p, i, t]
    r, n = xt.shape
    return np.ascontiguousarray(xt.reshape(r // P, P, n).transpose(1, 0, 2))


def _from_pit(y):
    # [P, R//128, N] -> [N, R]
    p, i, n = y.shape
    return y.transpose(2, 1, 0).reshape(n, i * p)


def _r2(n):
    return max(((n + 1) // 2) * 2, 8)


def kernel(hidden_states, router_w, base_w1, base_w2, exp_w1, exp_w2):
    x = np.ascontiguousarray(hidden_states.reshape(T, H), dtype=np.float32)
    top_w, top_i = _routing(x, np.asarray(router_w, dtype=np.float32))

    # per-expert token lists
    idx = []
    wts = []
    for e in range(E):
        rows, slots = np.nonzero(top_i == e)
        idx.append(rows)
        wts.append(top_w[rows, slots].astype(np.float32))
    n_e = np.array([len(r) for r in idx])

    # pair the biggest experts with the smallest: pair p -> cores {2p, 2p+1}
    order = np.argsort(-n_e, kind="stable")
    bigs = order[: E // 2]
    smalls = order[E // 2 :][::-1]
    SA = _r2((int(n_e[bigs].max()) + 1) // 2)
    SB = _r2((int(n_e[smalls].max()) + 1) // 2)
    C = SA + SB

    nc = _get_program(SA, SB)

    xT_c = np.ascontiguousarray(x.T).astype(NP_CDT)  # [H, T]

    b1_dev = _as_pkc(np.asarray(base_w1, np.float32).astype(NP_CDT), KA, FB)
    b2_dev = _as_pkc(np.asarray(base_w2, np.float32).astype(NP_CDT), KB, HB)
    w1_dev = [
        _as_pkc(np.asarray(exp_w1[e], np.float32).astype(NP_CDT), KA, FB)
        for e in range(E)
    ]
    w2_dev = [
        _as_pkc(np.asarray(exp_w2[e], np.float32).astype(NP_CDT), KB, HB)
        for e in range(E)
    ]

    # expert halves: expert e's tokens split between its two host cores
    parts = {}  # (pair, half) -> (big_rows, small_rows)
    for p in range(E // 2):
        ia, ib = idx[bigs[p]], idx[smalls[p]]
        ha, hb = (len(ia) + 1) // 2, (len(ib) + 1) // 2
        parts[(p, 0)] = (ia[:ha], ib[:hb])
        parts[(p, 1)] = (ia[ha:], ib[hb:])
    wmap = {}
    for p in range(E // 2):
        wa_, wb_ = wts[bigs[p]], wts[smalls[p]]
        ha, hb = (len(wa_) + 1) // 2, (len(wb_) + 1) // 2
        wmap[(p, 0)] = (wa_[:ha], wb_[:hb])
        wmap[(p, 1)] = (wa_[ha:], wb_[hb:])

    in_maps = []
    for c in range(E):
        p, half = c // 2, c % 2
        ra, rb = parts[(p, half)]
        va, vb = wmap[(p, half)]
        xg_full = np.zeros((H, C), dtype=NP_CDT)
        xg_full[:, : len(ra)] = xT_c[:, ra]
        xg_full[:, SA : SA + len(rb)] = xT_c[:, rb]
        wg_full = np.zeros((C,), dtype=np.float32)
        wg_full[: len(va)] = va
        wg_full[SA : SA + len(vb)] = vb
        in_maps.append(
            {
                "xg": _as_pit(xg_full),
                "wg": np.ascontiguousarray(np.broadcast_to(wg_full, (P, C))),
                "w1a": w1_dev[bigs[p]],
                "w1b": w1_dev[smalls[p]],
                "w2a": w2_dev[bigs[p]],
                "w2b": w2_dev[smalls[p]],
                "xb": _as_pit(xT_c[:, c * NB : (c + 1) * NB]),
                "b1": b1_dev,
                "b2": b2_dev,
            }
        )

    res = run_bass_kernel_spmd(nc, in_maps, core_ids=list(range(8)))

    out = np.empty((T, H), dtype=np.float32)
    for c in range(E):
        out[c * NB : (c + 1) * NB] = _from_pit(res.results[c]["ybase"])
    for c in range(E):
        p, half = c // 2, c % 2
        ra, rb = parts[(p, half)]
        ym = _from_pit(res.results[c]["ymoe"])
        if len(ra):
            out[ra] += BETA * ym[: len(ra)]
        if len(rb):
            out[rb] += BETA * ym[SA : SA + len(rb)]
    return out.reshape(B, S, H)
